# revision 8
# baseline (speedup 1.0000x reference)
"""Trainium2 Bass kernel for a single causal attention head (with the
faithful source bug: q = x @ W_key, W_query unused).

Full-input contract: kernel(x, W_key, W_query, W_value) -> [8, 2048, 128].
Sharding: data-parallel over batch B=8 across 8 NeuronCores (1 batch/core).

Per-core math (T=2048, C=1024, H=128):
    K = x @ W_key            (V = x @ W_value)
    S = K @ K.T * H**-0.5    (symmetric since q == k)
    out = softmax(causal(S)) @ V

Schedule (v2):
  - PE clock warmup: dummy wide matmuls on a DMA'd const scratch fill
    the input-DMA wait so projections start at max clock. Constants
    (identity / diag-mask / scratch / zero-bias) ride one early DMA;
    activation bias is passed as an AP so the const-arena never emits.
  - Projections accumulate K^T into 2 [P,1024] PSUM tiles and V^T into
    [P,1024]+2x[P,512] chasing the input DMA; kt casts split
    scalar/vector inline on the last c-tile.
  - Scores: upper triangle only (S symmetric since q==k); exp in
    1024-col chunks on ScalarE (~0.89 ns/col vs 1.23 at 512); causal
    mask is a post-exp multiply on diag tiles on GpSimd; denominators
    ride the AV matmul as a ones-column on V.
  - PSUM: 3x[P,1024] score/projection ring + 1 bank holding all three
    AV accumulation windows as 129-col subtiles + 1 transpose-scratch
    bank. AV updates interleave between score-chunk matmuls so their
    LDWEIGHTS hide under the 512-col streams; rows 14/15 hoisted.
  - Output: fp16 partition-major y[P, NT, H]; tiles 0-11 DMA in groups
    of 4 (one 128-descriptor trigger each), 12-15 individually so the
    tail pipelines; host reshapes/upcasts.
"""

import numpy as np

import concourse.bass as bass
import concourse.mybir as mybir
import concourse.tile as tile
from concourse import bacc, bass_utils


P = 128
T = 2048
C = 1024
H = 128
NT = T // P  # 16 seq tiles
NC = C // P  # 8 contraction tiles
NCORES = 8
SCALE = float(H) ** -0.5
F32 = mybir.dt.float32
FP16 = mybir.dt.float16
EXP = mybir.ActivationFunctionType.Exp
CW = 2 * P + 512  # const tensor width: ident | dmask | scratch


def build_module():
    nc = bacc.Bacc(
        "TRN2", target_bir_lowering=False, debug=False, num_devices=NCORES
    )
    xT_d = nc.dram_tensor("xT", [C, T], FP16, kind="ExternalInput").ap()
    w_d = nc.dram_tensor("W", [P, 2, NC, H], FP16, kind="ExternalInput").ap()
    c_d = nc.dram_tensor("C", [P, CW], FP16, kind="ExternalInput").ap()
    c32_d = nc.dram_tensor("C32", [P, 1], F32, kind="ExternalInput").ap()
    y_d = nc.dram_tensor("y", [P, NT, H], FP16, kind="ExternalOutput").ap()

    offs = []
    off = 0
    for j in range(NT):
        offs.append(off)
        off += (NT - j) * P
    e_width = off  # 136 * 128 = 17408

    with tile.TileContext(nc) as tc:
        with (
            tc.tile_pool(name="const", bufs=1) as const,
            tc.tile_pool(name="xt", bufs=8) as xt_pool,
            tc.tile_pool(name="kv", bufs=1) as kv,
            tc.tile_pool(name="e", bufs=1) as e_pool,
            tc.tile_pool(name="outp", bufs=4) as outp,
            tc.tile_pool(name="ps", bufs=1, space="PSUM") as ps,
        ):
            c_sb = const.tile([P, CW], FP16)
            nc.sync.dma_start(c_sb[:], c_d[:])
            w_sb = const.tile([P, 2, NC, H], FP16)
            nc.sync.dma_start(w_sb[:], w_d[:])
            wk_sb = w_sb[:, 0]
            wv_sb = w_sb[:, 1]
            ident = c_sb[:, 0:P]
            dmask = c_sb[:, P : 2 * P]
            scratch = c_sb[:, 2 * P : 2 * P + 512]

            kt_r = kv.tile([P, T], FP16)
            vt_sb = kv.tile([P, T], FP16)
            vaug = kv.tile([P, NT, P + 1], FP16)
            o_sb = kv.tile([P, NT, H], FP16)
            e_all = e_pool.tile([P, e_width], FP16)

            # PSUM: tag "big" 2x[P,1024] (4 banks) + tag "av" 3x[P,512]
            # (the AV accumulation windows — bank-granular, PSUM
            # accumulation groups cannot share a bank) + "vtr" 1 bank
            # for the V transposes = all 8 banks. During projections the
            # av and vtr banks double as V^T accumulators.
            kt_ps = [
                ps.tile([P, 1024], F32, tag="big", name=f"ktps{i}", bufs=2)
                for i in range(2)
            ]
            vt_ps = [
                ps.tile([P, 512], F32, tag="av", name=f"vtps{ch}", bufs=3)
                for ch in range(3)
            ] + [ps.tile([P, 512], F32, tag="vtr", name="vts", bufs=1)]

            # PE clock warmup: the tensor engine reaches max clock only
            # after ~3us of continuous execution; dummy wide matmuls on
            # the (first-DMA'd) const scratch fill the input-DMA wait.
            for d in range(8):
                nc.tensor.matmul(
                    kt_ps[0][:, 0:512],
                    scratch[:, 0:P],
                    scratch[:],
                    start=True,
                    stop=True,
                )

            CHW = 512
            for c in range(NC):
                xt_c = xt_pool.tile([P, T], FP16, tag="xt", name=f"xt{c}")
                eng = nc.sync if c % 2 else nc.scalar
                eng.dma_start(xt_c[:], xT_d[c * P : (c + 1) * P, :])
                last = c == NC - 1
                for ch in range(4):
                    rhs = xt_c[:, ch * CHW : (ch + 1) * CHW]
                    ksl = kt_ps[ch // 2][:, (ch % 2) * CHW : (ch % 2 + 1) * CHW]
                    nc.tensor.matmul(
                        ksl, wk_sb[:, c, :], rhs, start=(c == 0), stop=last
                    )
                    if last:
                        # kt casts split scalar/vector so scores row 0
                        # isn't gated on one serialized cast queue
                        sl = slice(ch * CHW, (ch + 1) * CHW)
                        if ch % 2 == 0:
                            nc.scalar.copy(kt_r[:, sl], ksl)
                        else:
                            nc.vector.tensor_copy(kt_r[:, sl], ksl)
                    nc.tensor.matmul(
                        vt_ps[ch],
                        wv_sb[:, c, :],
                        rhs,
                        start=(c == 0),
                        stop=last,
                    )
            for ch in range(4):
                sl = slice(ch * CHW, (ch + 1) * CHW)
                nc.vector.tensor_copy(vt_sb[:, sl], vt_ps[ch])

            # ACT exp-table warm after the scalar queue's DMA triggers;
            # zero-bias rides the C32 DMA so no const-arena memsets.
            zb = const.tile([P, 1], F32)
            nc.sync.dma_start(zb[:], c32_d[:])
            warm = const.tile([P, 1], F32)
            nc.scalar.activation(warm[:], zb[:], EXP, bias=zb[:])

            # vaug ones-column (denominator); VectorE runs it between
            # the vt casts and the first AV use.
            nc.vector.memset(vaug[:, :, P : P + 1], 1.0)

            NAV = P + 1

            def scores_row(j, fillers=()):
                # AV updates ride as fillers between score-chunk matmuls
                # so their LDWEIGHTS hide under the 512-col streams
                fillers = list(fillers)
                n_fill = len(fillers)
                emitted = 0
                b0 = j * P
                width = T - b0
                n_sub = -(-width // 512)
                si = 0
                pos = 0
                while pos < width:
                    w = min(1024, width - pos)
                    s_ps = ps.tile(
                        [P, 1024], F32, tag="big", name=f"sps{j}_{pos}", bufs=2
                    )
                    sp = 0
                    while sp < w:
                        sw = min(512, w - sp)
                        nc.tensor.matmul(
                            s_ps[:, sp : sp + sw],
                            kt_r[:, b0 : b0 + P],
                            kt_r[:, b0 + pos + sp : b0 + pos + sp + sw],
                            start=True,
                            stop=True,
                        )
                        sp += sw
                        si += 1
                        take = (n_fill * si) // n_sub - emitted
                        for th in fillers[emitted : emitted + take]:
                            th()
                        emitted += take
                    nc.scalar.activation(
                        e_all[:, offs[j] + pos : offs[j] + pos + w],
                        s_ps[:, :w],
                        EXP,
                        bias=zb[:],
                        scale=SCALE,
                    )
                    pos += w
                nc.gpsimd.tensor_mul(
                    e_all[:, offs[j] : offs[j] + P],
                    e_all[:, offs[j] : offs[j] + P],
                    dmask[:],
                )

            def transpose_v(j):
                vtr = ps.tile([P, 512], FP16, tag="vtr", name=f"vtr{j}", bufs=1)
                nc.tensor.transpose(
                    vtr[:, :P], vt_sb[:, j * P : (j + 1) * P], ident[:]
                )
                nc.vector.tensor_copy(vaug[:, j, 0:P], vtr[:, :P])

            WIN = 3
            av_banks = {}

            def new_av_bank(i):
                av_banks[i] = ps.tile(
                    [P, 512], F32, tag="av", name=f"avb{i}", bufs=3
                )

            def av_region(i):
                return av_banks[i][:, :NAV]

            def av_update(j, i, start, stop):
                eji = e_all[
                    :, offs[j] + (i - j) * P : offs[j] + (i - j + 1) * P
                ]
                nc.tensor.matmul(
                    av_region(i), eji, vaug[:, j, :], start=start, stop=stop
                )

            def normalize_out(i):
                av = av_region(i)
                recip = outp.tile([P, 1], F32, tag="recip", name=f"rcp{i}")
                nc.vector.reciprocal(recip[:], av[:, P : P + 1])
                if i >= NT - 2:
                    # ScalarE is idle after the (hoisted) last exps
                    nc.scalar.mul(o_sb[:, i], av[:, 0:P], recip[:])
                    nc.scalar.dma_start(y_d[:, i, :], o_sb[:, i, :])
                else:
                    nc.vector.tensor_scalar_mul(o_sb[:, i], av[:, 0:P], recip[:])
                    if i < 12 and i % 4 == 3:
                        g = i // 4
                        nc.sync.dma_start(
                            y_d[:, g * 4 : (g + 1) * 4, :],
                            o_sb[:, g * 4 : (g + 1) * 4, :],
                        )
                    elif i >= 12:
                        nc.sync.dma_start(y_d[:, i, :], o_sb[:, i, :])

            scores_row(0)
            transpose_v(0)
            for j in range(NT):
                # build this round's AV updates as thunks (window first,
                # then the newly activated column's catch-up)
                avs = []
                if j == 0:
                    for i in range(min(WIN, NT)):
                        new_av_bank(i)
                    for i in range(min(WIN, NT)):
                        avs.append(
                            lambda i=i: av_update(
                                0, i, start=True, stop=(i == 0)
                            )
                        )
                else:
                    for i in range(j, min(j + WIN - 1, NT)):
                        avs.append(
                            lambda i=i, j=j: av_update(
                                j, i, start=False, stop=(j == i)
                            )
                        )
                    act = j + WIN - 1
                    if act < NT:
                        new_av_bank(act)
                        for jc in range(j + 1):
                            avs.append(
                                lambda jc=jc, act=act: av_update(
                                    jc, act, start=(jc == 0), stop=False
                                )
                            )
                # rows 14/15 are hoisted to rounds 11/12 so the endgame
                # AV burst for the last columns never waits on ScalarE
                if j + 1 < NT - 2:
                    scores_row(j + 1, fillers=avs)
                    transpose_v(j + 1)
                    avs = []
                elif j + 1 < NT:
                    transpose_v(j + 1)
                if j == 11:
                    scores_row(14)
                if j == 12:
                    scores_row(15)
                for th in avs:
                    th()
                normalize_out(j)

    nc.compile()
    return nc


_NC_CACHE = None


def _get_module():
    global _NC_CACHE
    if _NC_CACHE is None:
        _NC_CACHE = build_module()
    return _NC_CACHE


def run(in_maps, trace=False, **kw):
    nc = _get_module()
    return bass_utils.run_bass_kernel_spmd(
        nc, in_maps, core_ids=list(range(NCORES)), trace=trace, **kw
    )


def make_in_maps(x, W_key, W_value):
    x = np.asarray(x, dtype=np.float32).astype(np.float16)
    xT = np.ascontiguousarray(x.transpose(0, 2, 1))
    wk = np.asarray(W_key, np.float32).astype(np.float16)
    wk = wk.reshape(NC, P, H).transpose(1, 0, 2)
    wv = np.asarray(W_value, np.float32).astype(np.float16)
    wv = wv.reshape(NC, P, H).transpose(1, 0, 2)
    w = np.ascontiguousarray(np.stack([wk, wv], axis=1))  # [P, 2, NC, H]
    r = np.arange(P)
    cst = np.zeros((P, CW), np.float16)
    cst[:, 0:P] = np.eye(P, dtype=np.float16)
    cst[:, P : 2 * P] = (r[None, :] >= r[:, None]).astype(np.float16)
    cst[:, 2 * P :] = 0.5
    c32 = np.zeros((P, 1), np.float32)
    return [
        {"xT": xT[b], "W": w, "C": cst, "C32": c32} for b in range(NCORES)
    ]


def postprocess(res):
    outs = []
    for b in range(NCORES):
        y = np.asarray(res.results[b]["y"])  # [P, NT, H] fp16
        outs.append(y.transpose(1, 0, 2).reshape(T, H).astype(np.float32))
    return np.stack(outs, axis=0)


def kernel(x, W_key, W_query, W_value):
    del W_query
    return postprocess(run(make_in_maps(x, W_key, W_value), trace=False))


# revision 30
# speedup vs baseline: 1.1110x; 1.1110x over previous
"""Trainium2 Bass kernel for a single causal attention head (with the
faithful source bug: q = x @ W_key, W_query unused).

Full-input contract: kernel(x, W_key, W_query, W_value) -> [8, 2048, 128].
Sharding: data-parallel over batch B=8 across 8 NeuronCores (1 batch/core).

Per-core math (T=2048, C=1024, H=128):
    K = x @ W_key            (V = x @ W_value)
    S = K @ K.T * H**-0.5    (symmetric since q == k)
    out = softmax(causal(S)) @ V

Schedule highlights:
  - PE clock warmup: dummy wide matmuls fill the input-DMA wait (the
    tensor engine runs ~2x slow for its first ~3us of execution);
    combined with the ACT-table warm moved after the scalar queue's
    DMA triggers, the projections start ~1.5us earlier AND at max
    clock (either change alone is neutral; together ~-1.7us).
  - Projections accumulate K^T/V^T over C in PSUM chasing the input
    DMA; kt casts split scalar/vector inline on the last c-tile.
  - Scores: upper triangle only (S symmetric since q==k); exp without
    max-subtraction in 512-col chunks on ScalarE; causal mask is a
    post-exp multiply on diag tiles; denominators ride the AV matmul
    as a ones-column on V. AV updates interleave between score-chunk
    matmuls so their LDWEIGHTS hide under the 512-col streams.
  - AV: WIN=3 sliding PSUM window over the shared 8-bank ring,
    one-round software pipeline; rows 14/15 scores+exp hoisted so the
    endgame never waits on ScalarE; last two output tiles
    normalize/DMA on the scalar queue.

Exec ~56-57.5us typical (device clock state varies +-20% between
invocations; exec_time includes ~7.2us fixed preamble and ~2.9us
teardown). Rel err 4e-4. fp8 fails the accuracy gate (simulated).
"""

import numpy as np

import concourse.bass as bass
import concourse.mybir as mybir
import concourse.tile as tile
from concourse import bacc, bass_utils
from concourse.masks import make_identity, make_upper_triangular


P = 128
T = 2048
C = 1024
H = 128
NT = T // P  # 16 seq tiles
NC = C // P  # 8 contraction tiles
NCORES = 8
SCALE = float(H) ** -0.5
F32 = mybir.dt.float32
FP16 = mybir.dt.float16
EXP = mybir.ActivationFunctionType.Exp


def build_module():
    nc = bacc.Bacc(
        "TRN2", target_bir_lowering=False, debug=False, num_devices=NCORES
    )
    xT_d = nc.dram_tensor("xT", [C, T], FP16, kind="ExternalInput").ap()
    w_d = nc.dram_tensor("W", [P, 2, NC, H], FP16, kind="ExternalInput").ap()
    c32_d = nc.dram_tensor("C32", [P, 1], F32, kind="ExternalInput").ap()
    y_d = nc.dram_tensor("y", [P, NT, H], FP16, kind="ExternalOutput").ap()

    offs = []
    off = 0
    for j in range(NT):
        offs.append(off)
        off += (NT - j) * P
    e_width = off  # 136 * 128 = 17408

    with tile.TileContext(nc) as tc:
        with (
            tc.tile_pool(name="const", bufs=1) as const,
            tc.tile_pool(name="xt", bufs=8) as xt_pool,
            tc.tile_pool(name="kv", bufs=1) as kv,
            tc.tile_pool(name="e", bufs=1) as e_pool,
            tc.tile_pool(name="outp", bufs=4) as outp,
            tc.tile_pool(name="ps", bufs=8, space="PSUM") as ps,
        ):
            w_sb = const.tile([P, 2, NC, H], FP16)
            nc.sync.dma_start(w_sb[:], w_d[:])
            wk_sb = w_sb[:, 0]
            wv_sb = w_sb[:, 1]

            ident_f = const.tile([P, P], F32)
            make_identity(nc, ident_f)
            dmask_f = const.tile([P, P], F32)
            make_upper_triangular(nc, dmask_f, val=1.0, diag=True)
            ident = const.tile([P, P], FP16)
            nc.vector.tensor_copy(ident[:], ident_f[:])
            dmask = const.tile([P, P], FP16)
            nc.vector.tensor_copy(dmask[:], dmask_f[:])
            ones_f = const.tile([P, 1], F32)
            nc.vector.memset(ones_f[:], 1.0)

            # PE clock warmup: the tensor engine reaches max clock only
            # after ~3us of continuous execution, so the first real
            # matmuls would otherwise run 2x slow. Fill the PE's
            # input-DMA wait with wide dummy matmuls on a scratch tile
            # (result lands in kt_ps[0], which the first real K matmul
            # resets via start=True).
            scratch = const.tile([P, 512], FP16)
            nc.vector.memset(scratch[:], 0.5)

            kt_r = kv.tile([P, T], FP16)
            vt_sb = kv.tile([P, T], FP16)
            vaug = kv.tile([P, NT, P + 1], FP16)
            o_sb = kv.tile([P, NT, H], FP16)
            e_all = e_pool.tile([P, e_width], FP16)

            CHW = 512
            CHN = T // CHW
            kt_ps = [
                ps.tile([P, 512], F32, tag="ps", name=f"ktps{ch}")
                for ch in range(CHN)
            ]
            vt_ps = [
                ps.tile([P, 512], F32, tag="ps", name=f"vtps{ch}")
                for ch in range(CHN)
            ]
            for d in range(8):
                nc.tensor.matmul(
                    kt_ps[0][:],
                    scratch[:, 0:P],
                    scratch[:],
                    start=True,
                    stop=True,
                )
            for c in range(NC):
                xt_c = xt_pool.tile([P, T], FP16, tag="xt", name=f"xt{c}")
                eng = nc.sync if c % 2 else nc.scalar
                eng.dma_start(xt_c[:], xT_d[c * P : (c + 1) * P, :])
                last = c == NC - 1
                for ch in range(CHN):
                    rhs = xt_c[:, ch * CHW : (ch + 1) * CHW]
                    nc.tensor.matmul(
                        kt_ps[ch][:],
                        wk_sb[:, c, :],
                        rhs,
                        start=(c == 0),
                        stop=last,
                    )
                    if last:
                        # kt casts split scalar/vector, emitted as each
                        # chunk's accumulation stops so scores row 0
                        # isn't gated on one serialized cast queue
                        sl = slice(ch * CHW, (ch + 1) * CHW)
                        if ch % 2 == 0:
                            nc.scalar.copy(kt_r[:, sl], kt_ps[ch][:])
                        else:
                            nc.vector.tensor_copy(kt_r[:, sl], kt_ps[ch][:])
                    nc.tensor.matmul(
                        vt_ps[ch][:],
                        wv_sb[:, c, :],
                        rhs,
                        start=(c == 0),
                        stop=last,
                    )
            for ch in range(CHN):
                sl = slice(ch * CHW, (ch + 1) * CHW)
                nc.vector.tensor_copy(vt_sb[:, sl], vt_ps[ch][:])

            # ACT exp-table warm: after the c-loop so the scalar queue's
            # xt DMA triggers run first (xt0's transfer starts ~1.5us
            # earlier); with the PE clock warmup the earlier projection
            # start is at full speed
            # The zero-bias rides a DMA'd AP — measured cheaper per
            # ACTIVATE than the const-arena float bias.
            zb = const.tile([P, 1], F32)
            nc.sync.dma_start(zb[:], c32_d[:])
            warm = const.tile([P, 1], F32)
            nc.scalar.activation(warm[:], zb[:], EXP, bias=zb[:])

            NAV = P + 1

            def scores_row(j, fillers=()):
                # AV updates ride as fillers between score-chunk matmuls
                # so their LDWEIGHTS (~97ns) hide under 512-col streams
                fillers = list(fillers)
                n_fill = len(fillers)
                emitted = 0
                b0 = j * P
                width = T - b0
                n_chunks = -(-width // 512)
                ci = 0
                pos = 0
                while pos < width:
                    w = min(512, width - pos)
                    s_ps = ps.tile([P, 512], F32, tag="ps", name=f"sps{j}_{pos}")
                    nc.tensor.matmul(
                        s_ps[:, :w],
                        kt_r[:, b0 : b0 + P],
                        kt_r[:, b0 + pos : b0 + pos + w],
                        start=True,
                        stop=True,
                    )
                    nc.scalar.activation(
                        e_all[:, offs[j] + pos : offs[j] + pos + w],
                        s_ps[:, :w],
                        EXP,
                        bias=zb[:],
                        scale=SCALE,
                    )
                    pos += w
                    ci += 1
                    take = (n_fill * ci) // n_chunks - emitted
                    for th in fillers[emitted : emitted + take]:
                        th()
                    emitted += take
                nc.vector.tensor_mul(
                    e_all[:, offs[j] : offs[j] + P],
                    e_all[:, offs[j] : offs[j] + P],
                    dmask[:],
                )

            nc.vector.memset(vaug[:, :, P : P + 1], 1.0)

            def transpose_v(j):
                vtr = ps.tile([P, 512], FP16, tag="ps", name=f"vtr{j}")
                nc.tensor.transpose(
                    vtr[:, :P], vt_sb[:, j * P : (j + 1) * P], ident[:]
                )
                nc.vector.tensor_copy(vaug[:, j, 0:P], vtr[:, :P])

            WIN = 3
            av_banks = {}

            def av_region(i):
                return av_banks[i][:, :NAV]

            def av_update(j, i, start, stop):
                eji = e_all[
                    :, offs[j] + (i - j) * P : offs[j] + (i - j + 1) * P
                ]
                nc.tensor.matmul(
                    av_region(i), eji, vaug[:, j, :], start=start, stop=stop
                )

            def normalize_out(i):
                # fp16 output staged in one partition-major SBUF buffer;
                # tiles 0-11 DMA in groups of 4 (one 128-descriptor
                # trigger each), 12-15 individually so the tail
                # pipelines. Host reshapes/upcasts.
                av = av_region(i)
                recip = outp.tile([P, 1], F32, tag="recip", name=f"rcp{i}")
                nc.vector.reciprocal(recip[:], av[:, P : P + 1])
                if i >= NT - 2:
                    # ScalarE is idle after the (hoisted) last exps; the
                    # triggers go on the (also-idle) sync queue so the
                    # scalar engine drains as soon as the muls finish
                    nc.scalar.mul(o_sb[:, i], av[:, 0:P], recip[:])
                    nc.sync.dma_start(y_d[:, i, :], o_sb[:, i, :])
                else:
                    nc.vector.tensor_scalar_mul(o_sb[:, i], av[:, 0:P], recip[:])
                    if i < 12 and i % 4 == 3:
                        g = i // 4
                        nc.sync.dma_start(
                            y_d[:, g * 4 : (g + 1) * 4, :],
                            o_sb[:, g * 4 : (g + 1) * 4, :],
                        )
                    elif i >= 12:
                        nc.sync.dma_start(y_d[:, i, :], o_sb[:, i, :])

            scores_row(0)
            transpose_v(0)
            for j in range(NT):
                # build this round's AV updates as thunks (window first,
                # then the newly activated column's catch-up)
                avs = []
                if j == 0:
                    for i in range(min(WIN, NT)):
                        av_banks[i] = ps.tile(
                            [P, 512], F32, tag="ps", name=f"avb{i}"
                        )
                    for i in range(min(WIN, NT)):
                        avs.append(
                            lambda i=i: av_update(
                                0, i, start=True, stop=(i == 0)
                            )
                        )
                else:
                    for i in range(j, min(j + WIN - 1, NT)):
                        avs.append(
                            lambda i=i, j=j: av_update(
                                j, i, start=False, stop=(j == i)
                            )
                        )
                    act = j + WIN - 1
                    if act < NT:
                        av_banks[act] = ps.tile(
                            [P, 512], F32, tag="ps", name=f"avb{act}"
                        )
                        for jc in range(j + 1):
                            avs.append(
                                lambda jc=jc, act=act: av_update(
                                    jc, act, start=(jc == 0), stop=False
                                )
                            )
                # rows 14/15 are hoisted to rounds 11/12 so the endgame
                # AV burst for the last columns never waits on ScalarE
                if j + 1 < NT - 2:
                    scores_row(j + 1, fillers=avs)
                    transpose_v(j + 1)
                    avs = []
                elif j + 1 < NT:
                    transpose_v(j + 1)
                if j == 11:
                    scores_row(14)
                if j == 12:
                    scores_row(15)
                for th in avs:
                    th()
                normalize_out(j)

    nc.compile()
    return nc


_NC_CACHE = None


def _get_module():
    global _NC_CACHE
    if _NC_CACHE is None:
        _NC_CACHE = build_module()
    return _NC_CACHE


def run(in_maps, trace=False, **kw):
    nc = _get_module()
    return bass_utils.run_bass_kernel_spmd(
        nc, in_maps, core_ids=list(range(NCORES)), trace=trace, **kw
    )


def make_in_maps(x, W_key, W_value):
    x = np.asarray(x, dtype=np.float32).astype(np.float16)
    xT = np.ascontiguousarray(x.transpose(0, 2, 1))
    wk = np.asarray(W_key, np.float32).astype(np.float16)
    wk = wk.reshape(NC, P, H).transpose(1, 0, 2)
    wv = np.asarray(W_value, np.float32).astype(np.float16)
    wv = wv.reshape(NC, P, H).transpose(1, 0, 2)
    w = np.ascontiguousarray(np.stack([wk, wv], axis=1))  # [P, 2, NC, H]
    c32 = np.zeros((P, 1), np.float32)
    return [{"xT": xT[b], "W": w, "C32": c32} for b in range(NCORES)]


def postprocess(res):
    outs = []
    for b in range(NCORES):
        y = np.asarray(res.results[b]["y"])  # [P, NT, H] fp16
        outs.append(y.transpose(1, 0, 2).reshape(T, H).astype(np.float32))
    return np.stack(outs, axis=0)


def kernel(x, W_key, W_query, W_value):
    del W_query
    return postprocess(run(make_in_maps(x, W_key, W_value), trace=False))

